# revision 1
# baseline (speedup 1.0000x reference)
"""Hashed-weight MLP (1024-4096-4096-32000, batch 2048) on 8 TRN2 NeuronCores.

Problem: h = relu(x @ W0); h = relu(h @ W1); out = h @ W2, where each
W_l[i, j] = hw_l[(a_l*i + b_l*j + c_l) % N_l] is a virtual (ROBE-Z hashed)
weight gathered from a small parameter vector.

Approach (column-parallel tensor parallelism on all three layers):
  * Since N_l is a power of two and b_l is odd, b_l is invertible mod N_l.
    Through the host-computed permuted table hw_bb[t] = hw[(b*t) % N], the
    virtual weight becomes ROW-CONTIGUOUS:
        W[i, j0+dj] = hw_bb[u_i + j0 + dj],   u_i = b^-1 (a*i + c) % N
    and row starts u_i form an arithmetic progression mod N with stride
    q = b^-1 a. A whole [in_dim x width] weight shard therefore materializes
    with a handful of 3-level strided DMAs (signed-residue ladder over q) —
    NO per-element gathers anywhere.
  * Each core owns a 1/8 column shard of every layer. Per-core shard offsets
    are absorbed into the host-side slice of hw_bb (keeping the device
    program SPMD-uniform). Activations stay transposed [features, batch].
  * GEMMs are bf16 with fp32 PSUM accumulation (max-rel-err ~4e-3).
    AllGathers after L0/L1 are chunked per batch-tile and hidden behind the
    next batch-tile's matmuls; weight materializations are pipelined on the
    scalar/sync HWDGE rings under compute.
"""
import sys
if "/opt/trn_rl_repo" not in sys.path:
    sys.path.insert(0, "/opt/trn_rl_repo")

import numpy as np
import ml_dtypes

import concourse.bass as bass
import concourse.bacc as bacc
import concourse.tile as tile
import concourse.mybir as mybir
from concourse.bass_utils import run_bass_kernel_spmd

N_CORES = 8
P = 128
NB = 512                      # moving free dim (batch tile)
BATCH = 2048
BT = BATCH // NB              # 4

LENS = [1024, 4096, 4096, 32000]
HASH_A = [9973, 10007, 10039]
HASH_B = [31013, 31019, 31039]
HASH_C = [557, 563, 569]
SIZES = [1048576, 1048576, 4194304]

JW = [512, 512, 4000]         # true per-core output shard width
WMAT = [512, 512, 4096]       # materialized width (L2 padded to 32 j-tiles)

BF = mybir.dt.bfloat16
F32 = mybir.dt.float32


def _plan_layer(l):
    N = SIZES[l]; a, b, ch = HASH_A[l], HASH_B[l], HASH_C[l]
    binv = pow(b, -1, N)
    q = (binv * a) % N
    u0 = (binv * ch) % N
    in_dim = LENS[l]; w = WMAT[l]
    best = None
    for k in range(1, min(in_dim, 600) + 1):
        r = (q * k) % N
        if r > N // 2:
            r -= N
        C1 = -(-in_dim // k)
        extra = q * (k - 1) + abs(r) * (C1 - 1)
        if best is None or extra < best[0]:
            best = (extra, k, C1, r)
    _, k, C1, r = best
    shift = max(0, -r * (C1 - 1))
    m_ext = shift + q * (k - 1) + max(r, 0) * (C1 - 1) + w + 64
    if l == 1:
        # L1 is materialized in two row-halves with an extra mod-N base
        # offset on the second half; cover it in the slice.
        m_ext += N
    return dict(N=N, a=a, b=b, ch=ch, q=q, u0=u0, k=k, C1=C1, r=r,
                shift=shift, m_ext=m_ext, rows=k * C1, in_dim=in_dim, w=w)


PLANS = [_plan_layer(l) for l in range(3)]
RG = [list(range(N_CORES))]


def build_nc():
    nc = bacc.Bacc("TRN2", target_bir_lowering=False, debug=False,
                   num_devices=N_CORES)

    xT_d = nc.dram_tensor("xT", [LENS[0], BATCH], BF, kind="ExternalInput").ap()
    hb = [nc.dram_tensor(f"hb{l}", [PLANS[l]["m_ext"]], BF,
                         kind="ExternalInput").ap() for l in range(3)]
    w_mat = [nc.dram_tensor(f"w{l}", [PLANS[l]["rows"], PLANS[l]["w"]], BF).ap()
             for l in range(1)]
    # W1 in two row-half tensors so layer 1 starts after half the
    # materialization; halves are driven from both HWDGE rings in parallel.
    HROWS = 43 * 48           # 2064 >= 2048 rows per half (k=43, C1_half=48)
    w1_h = [nc.dram_tensor(f"w1h{h}", [HROWS, 512], BF).ap() for h in range(2)]
    # L2 weight shard split into 4 j-group tensors so L2 can start as soon as
    # its first slab is materialized (dep tracking is per-tensor).
    w2_jg = [nc.dram_tensor(f"w2jg{g}", [PLANS[2]["rows"], 1024], BF).ap()
             for g in range(4)]
    # per-batch-tile activation chunks: local contribution + allgathered full
    h1c = [nc.dram_tensor(f"h1c{b}", [512, NB], BF).ap() for b in range(BT)]
    h1f = [nc.dram_tensor(f"h1f{b}", [4096, NB], BF, addr_space="Shared").ap()
           for b in range(BT)]
    h2c = [nc.dram_tensor(f"h2c{b}", [512, NB], BF).ap() for b in range(BT)]
    h2f = [nc.dram_tensor(f"h2f{b}", [4096, NB], BF, addr_space="Shared").ap()
           for b in range(BT)]
    out_d = nc.dram_tensor("outT", [4096, BATCH], F32, kind="ExternalOutput").ap()

    def matz_small(l):
        """Materialize W0. dim0 = a <=16-count block of the i1 axis: walrus
        splits a DMA across SDMA engine slots by the outermost dim only when
        its count is <= 16, so this shape fans out 16-wide."""
        pl = PLANS[l]
        q, k, C1, r, w = pl["q"], pl["k"], pl["C1"], pl["r"], pl["w"]
        off = pl["shift"]
        with nc.allow_non_contiguous_dma(reason="hash ladder materialization"):
            for k0 in range(0, k, 16):
                kc = min(16, k - k0)
                src = bass.AP(hb[l].tensor, off + q * k0,
                              [[q, kc], [r, C1], [1, w]])
                dst = bass.AP(w_mat[l].tensor, w * k0,
                              [[w, kc], [k * w, C1], [1, w]])
                nc.scalar.dma_start(out=dst, in_=src)

    def matz1_half(h):
        """Materialize W1 rows [2048h, 2048h+2064) from the periodic slice.
        Row i = 2048h + i0 + 43*i1; base offset (q*2048h) mod N."""
        pl = PLANS[1]
        q, k, r, w, N = pl["q"], pl["k"], pl["r"], pl["w"], pl["N"]
        C1h = 48
        off = pl["shift"] + (q * 2048 * h) % N
        eng = nc.scalar if h == 0 else nc.sync
        with nc.allow_non_contiguous_dma(reason="hash ladder materialization"):
            for k0 in range(0, k, 16):
                kc = min(16, k - k0)
                src = bass.AP(hb[1].tensor, off + q * k0,
                              [[q, kc], [r, C1h], [1, w]])
                dst = bass.AP(w1_h[h].tensor, w * k0,
                              [[w, kc], [k * w, C1h], [1, w]])
                eng.dma_start(out=dst, in_=src)

    def matz2(jgs):
        """Materialize L2 j-group slabs (2KB inner runs spread well)."""
        pl = PLANS[2]
        q, k, C1, r = pl["q"], pl["k"], pl["C1"], pl["r"]
        w = 1024
        nchunk = 4
        step = -(-C1 // nchunk)
        with nc.allow_non_contiguous_dma(reason="hash ladder materialization"):
            for g in jgs:
                for ci in range(nchunk):
                    c1a = ci * step
                    c1b = min(C1, c1a + step)
                    cnt = c1b - c1a
                    src = bass.AP(hb[2].tensor,
                                  pl["shift"] + g * 1024 + r * c1a,
                                  [[q, k], [r, cnt], [1, w]])
                    dst = bass.AP(w2_jg[g].tensor, k * w * c1a,
                                  [[w, k], [k * w, cnt], [1, w]])
                    nc.scalar.dma_start(out=dst, in_=src)

    with tile.TileContext(nc) as tc, \
         tc.tile_pool(name="ps", bufs=8, space="PSUM") as psp, \
         tc.tile_pool(name="slabA", bufs=1) as slabA, \
         tc.tile_pool(name="slabB", bufs=1) as slabB:
        # ---- L0 weight materialization + loads first: compute starts ASAP
        matz_small(0)

        with tc.tile_pool(name="l0", bufs=1) as l0p:
            xsb = [l0p.tile([P, BATCH], BF, name=f"xsb{kt}") for kt in range(8)]
            w0sb = [l0p.tile([P, 512], BF, name=f"w0sb{kt}") for kt in range(8)]
            h1sb = [l0p.tile([P, BATCH], BF, name=f"h1sb{j}") for j in range(4)]
            for kt in range(8):
                nc.sync.dma_start(out=xsb[kt][:], in_=xT_d[kt * P:(kt + 1) * P, :])
                nc.sync.dma_start(out=w0sb[kt][:], in_=w_mat[0][kt * P:(kt + 1) * P, :])

            # L1 materialization overlaps L0 compute (both HWDGE rings),
            # and L2's first j-group follows on the scalar ring.
            matz1_half(0)
            matz1_half(1)
            matz2([0])

            for b in range(BT):
                for j in range(4):
                    ps = psp.tile([P, NB], F32, tag="ps", name=f"ps0_{b}_{j}")
                    for kt in range(8):
                        nc.tensor.matmul(
                            out=ps[:],
                            lhsT=w0sb[kt][:, j * P:(j + 1) * P],
                            rhs=xsb[kt][:, b * NB:(b + 1) * NB],
                            start=(kt == 0), stop=(kt == 7))
                    nc.scalar.activation(out=h1sb[j][:, b * NB:(b + 1) * NB],
                                         in_=ps[:],
                                         func=mybir.ActivationFunctionType.Relu)
                for j in range(4):
                    nc.sync.dma_start(out=h1c[b][j * P:(j + 1) * P, :],
                                      in_=h1sb[j][:, b * NB:(b + 1) * NB])
                # chunked AllGather: hides behind the next batch-tile's matmuls
                nc.gpsimd.collective_compute(
                    "AllGather", mybir.AluOpType.bypass, replica_groups=RG,
                    ins=[h1c[b].opt()], outs=[h1f[b].opt()])

        # ---- Layer 1 ----
        with tc.tile_pool(name="l1w", bufs=1) as l1wp, \
             tc.tile_pool(name="l1r", bufs=6) as l1rp:
            w1sb = [l1wp.tile([P, 512], BF, name=f"w1sb{kt}") for kt in range(32)]
            h2sb = [l1wp.tile([P, NB], BF, name=f"h2sb{j}") for j in range(4)]
            for kt in range(32):
                h, lk = (0, kt) if kt < 16 else (1, kt - 16)
                nc.sync.dma_start(out=w1sb[kt][:],
                                  in_=w1_h[h][lk * P:(lk + 1) * P, :])

            # remaining L2 materialization overlaps L1 compute and AllGathers
            matz2([1, 2, 3])

            # prefetch L2 j-group 0 slab during layer 1
            slab0 = [slabA.tile([P, 1024], BF, tag=f"w2slab{kt}",
                                name=f"w2s_0_{kt}") for kt in range(32)]
            for kt in range(32):
                nc.scalar.dma_start(out=slab0[kt][:],
                                    in_=w2_jg[0][kt * P:(kt + 1) * P, :])

            for b in range(BT):
                pss = [psp.tile([P, NB], F32, tag="ps", name=f"ps1_{b}_{j}")
                       for j in range(4)]
                for kt in range(32):
                    rhs = l1rp.tile([P, NB], BF, tag="l1rhs", name=f"l1r_{b}_{kt}")
                    nc.sync.dma_start(out=rhs[:],
                                      in_=h1f[b][kt * P:(kt + 1) * P, :])
                    for j in range(4):
                        nc.tensor.matmul(
                            out=pss[j][:],
                            lhsT=w1sb[kt][:, j * P:(j + 1) * P],
                            rhs=rhs[:],
                            start=(kt == 0), stop=(kt == 31))
                for j in range(4):
                    nc.scalar.activation(out=h2sb[j][:],
                                         in_=pss[j][:],
                                         func=mybir.ActivationFunctionType.Relu)
                    nc.sync.dma_start(out=h2c[b][j * P:(j + 1) * P, :],
                                      in_=h2sb[j][:])
                nc.gpsimd.collective_compute(
                    "AllGather", mybir.AluOpType.bypass, replica_groups=RG,
                    ins=[h2c[b].opt()], outs=[h2f[b].opt()])

        # ---- Layer 2 (W2 slabbed by j-group, h2f streamed) ----
        with tc.tile_pool(name="l2r", bufs=6) as l2rp, \
             tc.tile_pool(name="l2o", bufs=4) as l2op:
            for jg in range(4):
                if jg == 0:
                    slab = slab0
                else:
                    pool = slabA if jg % 2 == 0 else slabB
                    slab = [pool.tile([P, 1024], BF, tag=f"w2slab{kt}",
                                      name=f"w2s_{jg}_{kt}") for kt in range(32)]
                    for kt in range(32):
                        nc.scalar.dma_start(
                            out=slab[kt][:],
                            in_=w2_jg[jg][kt * P:(kt + 1) * P, :])
                for b in range(BT):
                    pss = [psp.tile([P, NB], F32, tag="ps",
                                    name=f"ps2_{jg}_{b}_{j}") for j in range(8)]
                    for kt in range(32):
                        rhs = l2rp.tile([P, NB], BF, tag="l2rhs",
                                        name=f"l2r_{jg}_{b}_{kt}")
                        nc.sync.dma_start(out=rhs[:],
                                          in_=h2f[b][kt * P:(kt + 1) * P, :])
                        for j in range(8):
                            nc.tensor.matmul(
                                out=pss[j][:],
                                lhsT=slab[kt][:, j * P:(j + 1) * P],
                                rhs=rhs[:],
                                start=(kt == 0), stop=(kt == 31))
                    for j in range(8):
                        osb = l2op.tile([P, NB], F32, tag="l2out",
                                        name=f"l2o_{jg}_{b}_{j}")
                        nc.vector.tensor_copy(out=osb[:], in_=pss[j][:])
                        nc.scalar.dma_start(
                            out=out_d[(jg * 8 + j) * P:(jg * 8 + j + 1) * P,
                                      b * NB:(b + 1) * NB],
                            in_=osb[:])

    nc.compile()
    return nc


_NC_CACHE = None


def _get_nc():
    global _NC_CACHE
    if _NC_CACHE is None:
        _NC_CACHE = build_nc()
    return _NC_CACHE


def _prep_inputs(x, hw0, hw1, hw2):
    """Host prep: transpose x, build per-core periodic permuted-table slices."""
    x = np.asarray(x, np.float32)
    hws = [np.asarray(hw0, np.float32), np.asarray(hw1, np.float32),
           np.asarray(hw2, np.float32)]
    xT = np.ascontiguousarray(x.T).astype(ml_dtypes.bfloat16)

    per_core_hb = [[None] * 3 for _ in range(N_CORES)]
    for l in range(3):
        pl = PLANS[l]
        N, b = pl["N"], pl["b"]
        m_ext = pl["m_ext"]
        jw = JW[l]
        t0 = pl["u0"] - pl["shift"]          # core-0 slice start (in t-space)
        span = m_ext + (N_CORES - 1) * jw
        t = t0 + np.arange(span, dtype=np.int64)
        shared = hws[l][(b * t) % N].astype(ml_dtypes.bfloat16)
        for c in range(N_CORES):
            per_core_hb[c][l] = shared[c * jw: c * jw + m_ext]
    in_maps = []
    for c in range(N_CORES):
        in_maps.append({
            "xT": xT,
            "hb0": per_core_hb[c][0],
            "hb1": per_core_hb[c][1],
            "hb2": per_core_hb[c][2],
        })
    return in_maps


def kernel(x, hw0, hw1, hw2, trace=False):
    nc = _get_nc()
    in_maps = _prep_inputs(x, hw0, hw1, hw2)
    res = run_bass_kernel_spmd(nc, in_maps, list(range(N_CORES)), trace=trace)
    outs = [res.results[c]["outT"][:JW[2], :] for c in range(N_CORES)]
    full = np.concatenate(outs, axis=0)        # [32000, 2048]
    out = np.ascontiguousarray(full.T)         # [2048, 32000] fp32
    kernel.last_results = res
    return out

